# revision 28
# baseline (speedup 1.0000x reference)
"""Trainium2 Bass kernel for nn_CLIPModelShuffleAttentionPenultimateLayer.

Strategy (8 NeuronCores, SPMD):
  - The patch shuffle uses a FIXED jax PRNG key (42), so it is a
    data-independent pixel permutation. We precompute the gather index map
    once on host and apply it to x (pure data movement).
  - The dominant cost is feat_shuffle = xs(64,150528) @ W_clip(150528,1024),
    which is HBM-bound on streaming W_clip (~616MB = ~77MB/core). We shard
    the contraction (K) dim across the 8 cores. W tiles are cast f32->bf16
    on ScalarE/VectorE (hidden under the DMA stream); x is split into two
    bf16 operands (hi + lo) so the 2-pass matmul keeps near-f32 accuracy
    on the x side while staying under the DMA roofline.
  - The partial feat_shuffle is AllReduced in two uneven D-segments
    (768 + 256 cols): the first reduce overlaps the second segment's
    stream, leaving only a 64KB AllReduce exposed at the end.
  - The head is tensor-parallel over D. Per core: q/k/reduced/z column
    slices (z = fs @ (Wv@Wo) with the branch-mean folded into the
    attention weights, eliminating the separate attention-output matmuls),
    one tiny AllGather for the 3x3 score partials, then per-core fc/l2norm
    partial sums that the host combines during unsharding.
"""
import numpy as np

B = 64
C = 3
H = W_IMG = 224
P_SZ = 14
D = 1024
NCORES = 8
KFULL = C * H * W_IMG          # 150528
KC = KFULL // NCORES           # 18816 per core
KT = KC // 128                 # 147 k-tiles per core
CHUNK = 21                     # k-tiles per W DMA (147 = 7*21)
SEGS = [(0, 512), (512, 1024)]  # D segments for the overlapped reduces
DS = D // NCORES               # 128 col slice for q/k/wvo
RS = 256 // NCORES             # 32 col slice of W_red
EPS = 1e-5

_S3 = 1.0 / np.sqrt(3.0)
_DCT = np.array([
    [_S3, _S3, _S3],
    [np.sqrt(0.5), 0.0, -np.sqrt(0.5)],
    [np.sqrt(2.0 / 3.0) * 0.5, -np.sqrt(2.0 / 3.0), np.sqrt(2.0 / 3.0) * 0.5],
])
_MCOEF = _DCT.mean(axis=0)          # mean = sum_n MCOEF[n] * r_n
_ACOEF = _DCT - _MCOEF[None, :]     # xc_k = sum_n ACOEF[k,n] * r_n

_idx_cache = None
_prog_cache = None


def _shuffle_idx():
    """(B, KFULL) int gather map: shuffled_flat[b, j] = x_flat[b, idx[b, j]]."""
    global _idx_cache
    if _idx_cache is not None:
        return _idx_cache
    import jax
    import jax.numpy as jnp
    cpu = jax.devices("cpu")[0]
    with jax.default_device(cpu):
        b, c, h, w, p = B, C, H, W_IMG, P_SZ
        key = jax.random.key(42)
        x = jnp.broadcast_to(
            jnp.arange(c * h * w, dtype=jnp.int32).reshape(1, c, h, w),
            (b, c, h, w))
        nh, nw = h // p, w // p
        P = nh * nw
        patches = (x.reshape(b, c, nh, p, nw, p)
                   .transpose(0, 2, 4, 1, 3, 5).reshape(b, P, c, p, p))
        k1, k2, k3, k4 = jax.random.split(key, 4)
        perm = jax.random.permutation(k1, P)
        patches = patches[:, perm]
        k = jax.random.randint(k2, (b, P), 0, 4)
        rots = jnp.stack([patches,
                          jnp.rot90(patches, 1, axes=(3, 4)),
                          jnp.rot90(patches, 2, axes=(3, 4)),
                          jnp.rot90(patches, 3, axes=(3, 4))], axis=2)
        patches = jnp.take_along_axis(
            rots, k[:, :, None, None, None, None], axis=2)[:, :, 0]
        f1 = jax.random.uniform(k3, (b, P)) < 0.5
        f2 = jax.random.uniform(k4, (b, P)) < 0.5
        patches = jnp.where(f1[:, :, None, None, None],
                            patches[:, :, :, ::-1, :], patches)
        patches = jnp.where(f2[:, :, None, None, None],
                            patches[:, :, :, :, ::-1], patches)
        out = (patches.reshape(b, nh, nw, c, p, p)
               .transpose(0, 3, 1, 4, 2, 5).reshape(b, c, h, w))
        _idx_cache = np.asarray(out).reshape(b, -1).astype(np.int64)
    return _idx_cache


def _build_program():
    global _prog_cache
    if _prog_cache is not None:
        return _prog_cache
    import os
    from concourse import bacc, tile, mybir, masks

    f32 = mybir.dt.float32
    f32r = mybir.dt.float32r
    bf16 = mybir.dt.bfloat16
    ADD = mybir.AluOpType.add
    MUL = mybir.AluOpType.mult
    MAX = mybir.AluOpType.max
    Act = mybir.ActivationFunctionType

    nc = bacc.Bacc("TRN2", target_bir_lowering=False, debug=False,
                   num_devices=NCORES)

    xh_d = nc.dram_tensor("xh", [128, KT * B], bf16, kind="ExternalInput").ap()
    xl_d = nc.dram_tensor("xl", [128, KT * B], bf16, kind="ExternalInput").ap()
    w_d = nc.dram_tensor("w", [KC, D], f32, kind="ExternalInput").ap()
    fton_d = nc.dram_tensor("fton", [128, 8 * 128], f32r,
                            kind="ExternalInput").ap()
    headw_d = nc.dram_tensor("headw", [D, 416], f32r,
                             kind="ExternalInput").ap()
    fcx_d = nc.dram_tensor("fcx", [B, 257], f32, kind="ExternalInput").ap()
    out_d = nc.dram_tensor("out", [B, 4], f32, kind="ExternalOutput").ap()
    dbg_d = None
    if os.environ.get("KDBG"):
        dbg_d = nc.dram_tensor("dbg", [B, 1024], f32,
                               kind="ExternalOutput").ap()

    ar_in = [nc.dram_tensor(f"ar_in{s}", [B, e - b0], f32)
             for s, (b0, e) in enumerate(SEGS)]
    ar_out = [nc.dram_tensor(f"ar_out{s}", [B, e - b0], f32,
                             addr_space="Shared")
              for s, (b0, e) in enumerate(SEGS)]
    ag2_in = nc.dram_tensor("ag2_in", [B, 9], f32)
    ag2_out = nc.dram_tensor("ag2_out", [B * NCORES, 9], f32,
                             addr_space="Shared")

    rg = [list(range(NCORES))]

    with tile.TileContext(nc) as tc:
        with (
            tc.tile_pool(name="big", bufs=1) as big,
            tc.tile_pool(name="wp32", bufs=2) as wp32,
            tc.tile_pool(name="wp", bufs=2) as wp,
            tc.tile_pool(name="sb", bufs=1) as sb,
            tc.tile_pool(name="psA", bufs=1, space="PSUM") as psA,
            tc.tile_pool(name="psT", bufs=2, space="PSUM") as psT,
            tc.tile_pool(name="psP", bufs=3, space="PSUM") as psP,
        ):
            # ---------- inputs: x + small head inputs on the scalar HWDGE
            # queue so the sync queue carries only the W stream ------------
            xh = big.tile([128, KT * B], bf16, tag="xh")
            nc.scalar.dma_start(out=xh[:], in_=xh_d[:])
            xl = big.tile([128, KT * B], bf16, tag="xl")
            nc.scalar.dma_start(out=xl[:], in_=xl_d[:])
            headw = sb.tile([128, 8 * 416], f32r, tag="headw")
            nc.scalar.dma_start(
                out=headw[:].rearrange("p (t n) -> p t n", t=8),
                in_=headw_d.rearrange("(t p) n -> p t n", p=128),
            )
            fton = sb.tile([128, 8 * 128], f32r, tag="fton")
            nc.scalar.dma_start(out=fton[:], in_=fton_d[:])
            fcx = sb.tile([B, 257], f32, tag="fcx")
            nc.scalar.dma_start(out=fcx[:], in_=fcx_d[:])
            ident = sb.tile([B, B], f32, tag="ident")
            masks.make_identity(nc, ident[:])

            # ---------- branch 1/2 projections (hidden under stream) ------
            p12 = []
            for n in (0, 1):
                pp = psP.tile([B, 416], f32, tag="proj")
                for d in range(8):
                    nc.tensor.matmul(
                        pp[:, :],
                        lhsT=fton[:, d * 128 + n * 64: d * 128 + (n + 1) * 64],
                        rhs=headw[:, d * 416:(d + 1) * 416],
                        start=(d == 0), stop=(d == 7),
                    )
                ps = sb.tile([B, 416], f32, tag=f"p12_{n}")
                nc.vector.tensor_copy(ps[:], pp[:])
                p12.append(ps)

            # branch-0 projection accumulator (fed per segment after its AR)
            qkv0_ps = psP.tile([B, 416], f32, tag="proj")

            # ---------- big matmul over D-segments ------------------------
            for s, (b0, e0) in enumerate(SEGS):
                segw = e0 - b0
                feat_ps = psA.tile([B, segw], f32, tag="feat_ps")
                for c7 in range(KT // CHUNK):
                    wf = wp32.tile([128, CHUNK * segw], f32, tag="wf")
                    dma_eng = nc.sync if c7 % 2 == 0 else nc.scalar
                    dma_eng.dma_start(
                        out=wf[:].rearrange("p (t n) -> p t n", t=CHUNK),
                        in_=w_d[c7 * CHUNK * 128:(c7 + 1) * CHUNK * 128,
                                b0:e0]
                        .rearrange("(t p) n -> p t n", p=128),
                    )
                    wt = wp.tile([128, CHUNK * segw], bf16, tag="wt")
                    if c7 % 2 == 0:
                        nc.scalar.copy(wt[:], wf[:])
                    else:
                        nc.vector.tensor_copy(wt[:], wf[:])
                    for i in range(CHUNK):
                        kt = c7 * CHUNK + i
                        for nb in range(0, segw, 512):
                            nw_ = min(512, segw - nb)
                            nc.tensor.matmul(
                                feat_ps[:, nb:nb + nw_],
                                lhsT=xh[:, kt * B:(kt + 1) * B],
                                rhs=wt[:, i * segw + nb:i * segw + nb + nw_],
                                start=(kt == 0), stop=False,
                            )
                            nc.tensor.matmul(
                                feat_ps[:, nb:nb + nw_],
                                lhsT=xl[:, kt * B:(kt + 1) * B],
                                rhs=wt[:, i * segw + nb:i * segw + nb + nw_],
                                start=False, stop=(kt == KT - 1),
                            )
                part = sb.tile([B, segw], f32, tag=f"part{s}")
                nc.vector.tensor_copy(part[:], feat_ps[:])
                nc.gpsimd.dma_start(out=ar_in[s][:], in_=part[:])
                nc.gpsimd.collective_compute(
                    "AllReduce", ADD, replica_groups=rg,
                    ins=[ar_in[s][:]], outs=[ar_out[s][:]],
                )
                # read back the summed segment, transpose, project branch 0
                fseg = sb.tile([B, segw], f32, tag=f"fseg{s}")
                nc.sync.dma_start(out=fseg[:], in_=ar_out[s][:])
                nds = segw // 128
                tp_ps = psT.tile([128, nds * 64], f32, tag="tp")
                for dd in range(nds):
                    nc.tensor.transpose(
                        tp_ps[:, dd * 64:(dd + 1) * 64],
                        fseg[:, dd * 128:(dd + 1) * 128], ident[:])
                pfT = sb.tile([128, nds * B], f32r, tag=f"pfT{s}")
                nc.vector.tensor_copy(pfT[:], tp_ps[:])
                for dd in range(nds):
                    d = b0 // 128 + dd
                    nc.tensor.matmul(
                        qkv0_ps[:, :],
                        lhsT=pfT[:, dd * B:(dd + 1) * B],
                        rhs=headw[:, d * 416:(d + 1) * 416],
                        start=(d == 0), stop=(d == 7),
                    )
            qkv0 = sb.tile([B, 416], f32, tag="qkv0")
            nc.vector.tensor_copy(qkv0[:], qkv0_ps[:])

            qkv = [qkv0, p12[0], p12[1]]
            # col layout per branch: q 0:128 | k 128:256 | red 256:288 | z 288:416

            # ---------- score partials (critical path to AG2) ----------
            ag2s = sb.tile([B, 9], f32, tag="ag2s")
            scr = sb.tile([B, 128], f32, tag="scr")
            scr9 = sb.tile([B, 9 * 128], f32, tag="scr9")
            for n in range(3):
                for m in range(3):
                    j = n * 3 + m
                    nc.vector.scalar_tensor_tensor(
                        out=scr9[:, j * 128:(j + 1) * 128],
                        in0=qkv[n][:, 0:128],
                        scalar=1.0 / 32.0, in1=qkv[m][:, 128:256],
                        op0=MUL, op1=MUL)
            nc.vector.tensor_reduce(
                ag2s[:].rearrange("p (j o) -> p j o", o=1),
                scr9[:].rearrange("p (j d) -> p j d", j=9),
                axis=mybir.AxisListType.X, op=ADD)

            nc.gpsimd.dma_start(out=ag2_in[:], in_=ag2s[:])
            nc.gpsimd.collective_compute(
                "AllGather", mybir.AluOpType.bypass, replica_groups=rg,
                ins=[ag2_in[:]], outs=[ag2_out[:]],
            )

            # ---------- moment branch (overlaps the AllGather) ----------
            mom = sb.tile([B, 128], f32, tag="mom")
            r_ = [qkv[n][:, 256:288] for n in range(3)]

            def lincomb(dst, coef):
                nc.vector.tensor_scalar_mul(dst, r_[0], float(coef[0]))
                nc.vector.scalar_tensor_tensor(
                    out=dst, in0=r_[1], scalar=float(coef[1]), in1=dst,
                    op0=MUL, op1=ADD)
                nc.vector.scalar_tensor_tensor(
                    out=dst, in0=r_[2], scalar=float(coef[2]), in1=dst,
                    op0=MUL, op1=ADD)

            lincomb(mom[:, 0:32], _MCOEF)
            xc = []
            for kk in range(3):
                t = sb.tile([B, 32], f32, tag=f"xc{kk}")
                lincomb(t[:], _ACOEF[kk])
                xc.append(t)
            sumsq = sb.tile([B, 32], f32, tag="sumsq")
            sum3 = sb.tile([B, 32], f32, tag="sum3")
            sum4 = sb.tile([B, 32], f32, tag="sum4")
            tmp = sb.tile([B, 32], f32, tag="mtmp")
            sq = []
            for kk in range(3):
                s2 = sb.tile([B, 32], f32, tag=f"sq{kk}")
                nc.scalar.square(s2[:], xc[kk][:])
                sq.append(s2)
            nc.vector.tensor_add(sumsq[:], sq[0][:], sq[1][:])
            nc.vector.tensor_add(sumsq[:], sumsq[:], sq[2][:])
            nc.vector.tensor_mul(sum3[:], sq[0][:], xc[0][:])
            nc.vector.tensor_mul(tmp[:], sq[1][:], xc[1][:])
            nc.vector.tensor_add(sum3[:], sum3[:], tmp[:])
            nc.vector.tensor_mul(tmp[:], sq[2][:], xc[2][:])
            nc.vector.tensor_add(sum3[:], sum3[:], tmp[:])
            nc.scalar.square(sum4[:], sq[0][:])
            nc.scalar.square(tmp[:], sq[1][:])
            nc.vector.tensor_add(sum4[:], sum4[:], tmp[:])
            nc.scalar.square(tmp[:], sq[2][:])
            nc.vector.tensor_add(sum4[:], sum4[:], tmp[:])

            nc.vector.tensor_scalar_mul(mom[:, 32:64], sumsq[:], 1.0 / 3.0)
            veps = sb.tile([B, 32], f32, tag="veps")
            nc.vector.tensor_scalar(veps[:], sumsq[:], 1.0 / 3.0, EPS,
                                    MUL, ADD)
            std = sb.tile([B, 32], f32, tag="std")
            nc.scalar.sqrt(std[:], veps[:])
            d3 = sb.tile([B, 32], f32, tag="d3")
            nc.vector.tensor_mul(d3[:], std[:], veps[:])
            nc.vector.tensor_scalar(d3[:], d3[:], 3.0, 3.0 * EPS, MUL, ADD)
            rec = sb.tile([B, 32], f32, tag="rec")
            nc.vector.reciprocal(rec[:], d3[:])
            nc.vector.tensor_mul(mom[:, 64:96], sum3[:], rec[:])
            d4 = sb.tile([B, 32], f32, tag="d4")
            nc.vector.tensor_mul(d4[:], veps[:], veps[:])
            nc.vector.tensor_scalar(d4[:], d4[:], 3.0, 3.0 * EPS, MUL, ADD)
            nc.vector.reciprocal(rec[:], d4[:])
            nc.vector.tensor_mul(mom[:, 96:128], sum4[:], rec[:])

            # ---------- gather scores, softmax, attention mix ----------
            ag2g = sb.tile([B, 9 * NCORES], f32, tag="ag2g")
            nc.sync.dma_start(
                out=ag2g[:].rearrange("p (f r) -> p f r", r=NCORES),
                in_=ag2_out.rearrange("(r p) f -> p f r", p=B),
            )
            red2 = sb.tile([B, 9], f32, tag="red2")
            nc.vector.tensor_reduce(
                red2[:],
                ag2g[:].rearrange("p (f r) -> p f r", r=NCORES),
                axis=mybir.AxisListType.X, op=ADD)

            wt3 = sb.tile([B, 9], f32, tag="wt3")
            negmx = sb.tile([B, 3], f32, tag="negmx")
            sume = sb.tile([B, 3], f32, tag="sume")
            rn = sb.tile([B, 3], f32, tag="rn")
            etile = sb.tile([B, 9], f32, tag="etile")
            nc.vector.tensor_reduce(negmx[:],
                                    red2[:].rearrange("p (n m) -> p n m", n=3),
                                    axis=mybir.AxisListType.X, op=MAX)
            nc.vector.tensor_scalar_mul(negmx[:], negmx[:], -1.0)
            for n in range(3):
                nc.scalar.activation(etile[:, n * 3:(n + 1) * 3],
                                     red2[:, n * 3:(n + 1) * 3], Act.Exp,
                                     bias=negmx[:, n:n + 1],
                                     scale=1.0,
                                     accum_out=sume[:, n:n + 1])
            nc.vector.tensor_scalar_mul(sume[:], sume[:], 3.0)
            nc.vector.reciprocal(rn[:], sume[:])
            for n in range(3):
                nc.vector.tensor_scalar(wt3[:, n * 3:(n + 1) * 3],
                                        etile[:, n * 3:(n + 1) * 3],
                                        rn[:, n:n + 1], None, MUL)
            wbar = sb.tile([B, 3], f32, tag="wbar")
            nc.vector.tensor_reduce(
                wbar[:],
                wt3[:].rearrange("p (n m) -> p m n", n=3),
                axis=mybir.AxisListType.X, op=ADD)

            ao = sb.tile([B, 128], f32, tag="ao")
            nc.vector.tensor_scalar(ao[:], qkv[0][:, 288:416],
                                    wbar[:, 0:1], None, MUL)
            nc.vector.scalar_tensor_tensor(
                out=ao[:], in0=qkv[1][:, 288:416], scalar=wbar[:, 1:2],
                in1=ao[:], op0=MUL, op1=ADD)
            nc.vector.scalar_tensor_tensor(
                out=ao[:], in0=qkv[2][:, 288:416], scalar=wbar[:, 2:3],
                in1=ao[:], op0=MUL, op1=ADD)

            # ---------- partial sums for host-side fc/l2norm ----------
            outp = sb.tile([B, 4], f32, tag="outp")
            scr2 = sb.tile([B, 128], f32, tag="scr2")
            nc.vector.tensor_mul(scr2[:], ao[:], fcx[:, 0:128])
            nc.vector.tensor_reduce(outp[:, 0:1], scr2[:],
                                    axis=mybir.AxisListType.X, op=ADD)
            nc.scalar.activation(scr2[:], ao[:], Act.Square,
                                 accum_out=outp[:, 1:2])
            nc.vector.tensor_mul(scr[:], mom[:], fcx[:, 128:256])
            nc.vector.tensor_reduce(outp[:, 2:3], scr[:],
                                    axis=mybir.AxisListType.X, op=ADD)
            nc.scalar.activation(scr[:], mom[:], Act.Square,
                                 accum_out=outp[:, 3:4])
            nc.sync.dma_start(out=out_d[:], in_=outp[:])

            if dbg_d is not None:
                nc.sync.dma_start(out=dbg_d[:, 0:416], in_=qkv0[:])
                nc.sync.dma_start(out=dbg_d[:, 416:425], in_=red2[:])
                nc.sync.dma_start(out=dbg_d[:, 425:428], in_=wbar[:])
                nc.sync.dma_start(out=dbg_d[:, 428:556], in_=ao[:])

    nc.compile()
    _prog_cache = nc
    return nc


def _make_in_maps(x, feat_orig, noise_feat, W_clip, W_red, Wq, Wk, Wv, Wo,
                  fc_w, fc_b):
    import ml_dtypes
    bf = ml_dtypes.bfloat16

    x = np.asarray(x, dtype=np.float32)
    feat_orig = np.asarray(feat_orig, dtype=np.float32)
    noise_feat = np.asarray(noise_feat, dtype=np.float32)
    W_clip = np.asarray(W_clip, dtype=np.float32)
    W_red = np.asarray(W_red, dtype=np.float32)
    fc_w = np.asarray(fc_w, dtype=np.float32)

    idx = _shuffle_idx()
    xs = np.take_along_axis(x.reshape(B, -1), idx, axis=1)  # (64, 150528)
    xs4 = xs.reshape(B, NCORES, KT, 128)
    xsT = np.ascontiguousarray(xs4.transpose(1, 3, 2, 0)).reshape(
        NCORES, 128, KT * B)
    xh = xsT.astype(bf)
    xl = (xsT - xh.astype(np.float32)).astype(bf)

    fton = np.empty((128, 8, 128), dtype=np.float32)
    fton[:, :, 0:64] = feat_orig.T.reshape(8, 128, B).transpose(1, 0, 2)
    fton[:, :, 64:128] = noise_feat.T.reshape(8, 128, B).transpose(1, 0, 2)
    fton = fton.reshape(128, 8 * 128)

    Wvo = (np.asarray(Wv, np.float64) @ np.asarray(Wo, np.float64)).astype(
        np.float32)

    in_maps = []
    for c in range(NCORES):
        headw = np.concatenate([
            np.asarray(Wq, np.float32)[:, c * DS:(c + 1) * DS],
            np.asarray(Wk, np.float32)[:, c * DS:(c + 1) * DS],
            W_red[:, c * RS:(c + 1) * RS],
            Wvo[:, c * DS:(c + 1) * DS],
        ], axis=1)  # (1024, 416) f32 (consumed as f32r)
        fcwa = fc_w[c * DS:(c + 1) * DS, 0]
        fcwb = fc_w[D + np.arange(4)[:, None] * 256
                    + c * RS + np.arange(RS)[None, :], 0].reshape(-1)
        fcx = np.empty((B, 257), dtype=np.float32)
        fcx[:, 0:128] = fcwa[None, :]
        fcx[:, 128:256] = fcwb[None, :]
        fcx[:, 256] = 0.0
        in_maps.append({
            "xh": xh[c],
            "xl": xl[c],
            "w": W_clip[c * KC:(c + 1) * KC],
            "fton": fton,
            "headw": headw,
            "fcx": fcx,
        })
    return in_maps


def kernel(**inputs) -> np.ndarray:
    from concourse.bass_utils import run_bass_kernel_spmd
    fc_b = np.asarray(inputs["fc_b"], dtype=np.float32)
    in_maps = _make_in_maps(**inputs)
    nc = _build_program()
    for attempt in range(3):
        res = run_bass_kernel_spmd(nc, in_maps, core_ids=list(range(NCORES)))
        # unshard: sum per-core fc/l2norm partials, finish the scalar math
        P = np.zeros((B, 4), dtype=np.float64)
        for c in range(NCORES):
            P += res.results[c]["out"].astype(np.float64)
        nA = np.maximum(np.sqrt(np.maximum(P[:, 1], 0.0)), 1e-12)
        nB = np.maximum(np.sqrt(np.maximum(P[:, 3], 0.0)), 1e-12)
        logits = P[:, 0] / nA + P[:, 2] / nB + fc_b[0]
        # sanity gate: partials bounded, norms positive, logits finite/small
        ok = (np.all(np.isfinite(P)) and np.all(np.abs(P) < 1e8)
              and np.all(P[:, 1] > 0) and np.all(P[:, 3] > 0)
              and np.all(np.isfinite(logits))
              and np.all(np.abs(logits) < 1e4))
        if ok:
            break
    return logits.astype(np.float32).reshape(B, 1)


# revision 29
# speedup vs baseline: 1.1188x; 1.1188x over previous
"""Trainium2 Bass kernel for nn_CLIPModelShuffleAttentionPenultimateLayer.

Strategy (8 NeuronCores, SPMD):
  - The patch shuffle uses a FIXED jax PRNG key (42), so it is a
    data-independent pixel permutation. We precompute the gather index map
    once on host and apply it to x (pure data movement).
  - The dominant cost is feat_shuffle = xs(64,150528) @ W_clip(150528,1024),
    which is HBM-bound on streaming W_clip (~616MB = ~77MB/core). We shard
    the contraction (K) dim across the 8 cores. W tiles are cast f32->bf16
    on ScalarE/VectorE (hidden under the DMA stream); x is split into two
    bf16 operands (hi + lo) so the 2-pass matmul keeps near-f32 accuracy
    on the x side while staying under the DMA roofline.
  - The partial feat_shuffle is AllReduced in two uneven D-segments
    (768 + 256 cols): the first reduce overlaps the second segment's
    stream, leaving only a 64KB AllReduce exposed at the end.
  - The head is tensor-parallel over D. Per core: q/k/reduced/z column
    slices (z = fs @ (Wv@Wo) with the branch-mean folded into the
    attention weights, eliminating the separate attention-output matmuls),
    one tiny AllGather for the 3x3 score partials, then per-core fc/l2norm
    partial sums that the host combines during unsharding.
"""
import numpy as np

B = 64
C = 3
H = W_IMG = 224
P_SZ = 14
D = 1024
NCORES = 8
KFULL = C * H * W_IMG          # 150528
KC = KFULL // NCORES           # 18816 per core
KT = KC // 128                 # 147 k-tiles per core
CHUNK = 7                      # k-tiles per W DMA (147 = 21*7)
SEGS = [(0, 768), (768, 1024)]  # D segments for the overlapped reduces
DS = D // NCORES               # 128 col slice for q/k/wvo
RS = 256 // NCORES             # 32 col slice of W_red
EPS = 1e-5

_S3 = 1.0 / np.sqrt(3.0)
_DCT = np.array([
    [_S3, _S3, _S3],
    [np.sqrt(0.5), 0.0, -np.sqrt(0.5)],
    [np.sqrt(2.0 / 3.0) * 0.5, -np.sqrt(2.0 / 3.0), np.sqrt(2.0 / 3.0) * 0.5],
])
_MCOEF = _DCT.mean(axis=0)          # mean = sum_n MCOEF[n] * r_n
_ACOEF = _DCT - _MCOEF[None, :]     # xc_k = sum_n ACOEF[k,n] * r_n

_idx_cache = None
_prog_cache = None


def _shuffle_idx():
    """(B, KFULL) int gather map: shuffled_flat[b, j] = x_flat[b, idx[b, j]]."""
    global _idx_cache
    if _idx_cache is not None:
        return _idx_cache
    import jax
    import jax.numpy as jnp
    cpu = jax.devices("cpu")[0]
    with jax.default_device(cpu):
        b, c, h, w, p = B, C, H, W_IMG, P_SZ
        key = jax.random.key(42)
        x = jnp.broadcast_to(
            jnp.arange(c * h * w, dtype=jnp.int32).reshape(1, c, h, w),
            (b, c, h, w))
        nh, nw = h // p, w // p
        P = nh * nw
        patches = (x.reshape(b, c, nh, p, nw, p)
                   .transpose(0, 2, 4, 1, 3, 5).reshape(b, P, c, p, p))
        k1, k2, k3, k4 = jax.random.split(key, 4)
        perm = jax.random.permutation(k1, P)
        patches = patches[:, perm]
        k = jax.random.randint(k2, (b, P), 0, 4)
        rots = jnp.stack([patches,
                          jnp.rot90(patches, 1, axes=(3, 4)),
                          jnp.rot90(patches, 2, axes=(3, 4)),
                          jnp.rot90(patches, 3, axes=(3, 4))], axis=2)
        patches = jnp.take_along_axis(
            rots, k[:, :, None, None, None, None], axis=2)[:, :, 0]
        f1 = jax.random.uniform(k3, (b, P)) < 0.5
        f2 = jax.random.uniform(k4, (b, P)) < 0.5
        patches = jnp.where(f1[:, :, None, None, None],
                            patches[:, :, :, ::-1, :], patches)
        patches = jnp.where(f2[:, :, None, None, None],
                            patches[:, :, :, :, ::-1], patches)
        out = (patches.reshape(b, nh, nw, c, p, p)
               .transpose(0, 3, 1, 4, 2, 5).reshape(b, c, h, w))
        _idx_cache = np.asarray(out).reshape(b, -1).astype(np.int64)
    return _idx_cache


def _build_program():
    global _prog_cache
    if _prog_cache is not None:
        return _prog_cache
    import os
    from concourse import bacc, tile, mybir, masks

    f32 = mybir.dt.float32
    f32r = mybir.dt.float32r
    bf16 = mybir.dt.bfloat16
    ADD = mybir.AluOpType.add
    MUL = mybir.AluOpType.mult
    MAX = mybir.AluOpType.max
    Act = mybir.ActivationFunctionType

    nc = bacc.Bacc("TRN2", target_bir_lowering=False, debug=False,
                   num_devices=NCORES)

    xh_d = nc.dram_tensor("xh", [128, KT * B], bf16, kind="ExternalInput").ap()
    xl_d = nc.dram_tensor("xl", [128, KT * B], bf16, kind="ExternalInput").ap()
    w_d = nc.dram_tensor("w", [KC, D], f32, kind="ExternalInput").ap()
    fton_d = nc.dram_tensor("fton", [128, 8 * 128], f32r,
                            kind="ExternalInput").ap()
    headw_d = nc.dram_tensor("headw", [D, 416], f32r,
                             kind="ExternalInput").ap()
    fcx_d = nc.dram_tensor("fcx", [B, 257], f32, kind="ExternalInput").ap()
    out_d = nc.dram_tensor("out", [B, 4], f32, kind="ExternalOutput").ap()
    dbg_d = None
    if os.environ.get("KDBG"):
        dbg_d = nc.dram_tensor("dbg", [B, 1024], f32,
                               kind="ExternalOutput").ap()

    ar_in = [nc.dram_tensor(f"ar_in{s}", [B, e - b0], f32)
             for s, (b0, e) in enumerate(SEGS)]
    ar_out = [nc.dram_tensor(f"ar_out{s}", [B, e - b0], f32,
                             addr_space="Shared")
              for s, (b0, e) in enumerate(SEGS)]
    ag2_in = nc.dram_tensor("ag2_in", [B, 9], f32)
    ag2_out = nc.dram_tensor("ag2_out", [B * NCORES, 9], f32,
                             addr_space="Shared")

    rg = [list(range(NCORES))]

    with tile.TileContext(nc) as tc:
        with (
            tc.tile_pool(name="big", bufs=1) as big,
            tc.tile_pool(name="wp32", bufs=3) as wp32,
            tc.tile_pool(name="wp", bufs=4) as wp,
            tc.tile_pool(name="sb", bufs=1) as sb,
            tc.tile_pool(name="psA", bufs=1, space="PSUM") as psA,
            tc.tile_pool(name="psT", bufs=2, space="PSUM") as psT,
            tc.tile_pool(name="psP", bufs=3, space="PSUM") as psP,
        ):
            # ---------- inputs: x + small head inputs on the scalar HWDGE
            # queue so the sync queue carries only the W stream ------------
            xh = big.tile([128, KT * B], bf16, tag="xh")
            nc.scalar.dma_start(out=xh[:], in_=xh_d[:])
            xl = big.tile([128, KT * B], bf16, tag="xl")
            nc.scalar.dma_start(out=xl[:], in_=xl_d[:])
            headw = sb.tile([128, 8 * 416], f32r, tag="headw")
            nc.scalar.dma_start(
                out=headw[:].rearrange("p (t n) -> p t n", t=8),
                in_=headw_d.rearrange("(t p) n -> p t n", p=128),
            )
            fton = sb.tile([128, 8 * 128], f32r, tag="fton")
            nc.scalar.dma_start(out=fton[:], in_=fton_d[:])
            fcx = sb.tile([B, 257], f32, tag="fcx")
            nc.scalar.dma_start(out=fcx[:], in_=fcx_d[:])
            ident = sb.tile([B, B], f32, tag="ident")
            masks.make_identity(nc, ident[:])

            # ---------- branch 1/2 projections (hidden under stream) ------
            p12 = []
            for n in (0, 1):
                pp = psP.tile([B, 416], f32, tag="proj")
                for d in range(8):
                    nc.tensor.matmul(
                        pp[:, :],
                        lhsT=fton[:, d * 128 + n * 64: d * 128 + (n + 1) * 64],
                        rhs=headw[:, d * 416:(d + 1) * 416],
                        start=(d == 0), stop=(d == 7),
                    )
                ps = sb.tile([B, 416], f32, tag=f"p12_{n}")
                nc.vector.tensor_copy(ps[:], pp[:])
                p12.append(ps)

            # branch-0 projection accumulator (fed per segment after its AR)
            qkv0_ps = psP.tile([B, 416], f32, tag="proj")

            # ---------- big matmul over D-segments ------------------------
            for s, (b0, e0) in enumerate(SEGS):
                segw = e0 - b0
                feat_ps = psA.tile([B, segw], f32, tag="feat_ps")
                for c7 in range(KT // CHUNK):
                    wf = wp32.tile([128, CHUNK * segw], f32, tag="wf")
                    dma_eng = nc.sync if c7 % 2 == 0 else nc.scalar
                    dma_eng.dma_start(
                        out=wf[:].rearrange("p (t n) -> p t n", t=CHUNK),
                        in_=w_d[c7 * CHUNK * 128:(c7 + 1) * CHUNK * 128,
                                b0:e0]
                        .rearrange("(t p) n -> p t n", p=128),
                    )
                    wt = wp.tile([128, CHUNK * segw], bf16, tag="wt")
                    if c7 % 2 == 0:
                        nc.scalar.copy(wt[:], wf[:])
                    else:
                        nc.vector.tensor_copy(wt[:], wf[:])
                    for i in range(CHUNK):
                        kt = c7 * CHUNK + i
                        for nb in range(0, segw, 512):
                            nw_ = min(512, segw - nb)
                            nc.tensor.matmul(
                                feat_ps[:, nb:nb + nw_],
                                lhsT=xh[:, kt * B:(kt + 1) * B],
                                rhs=wt[:, i * segw + nb:i * segw + nb + nw_],
                                start=(kt == 0), stop=False,
                            )
                            nc.tensor.matmul(
                                feat_ps[:, nb:nb + nw_],
                                lhsT=xl[:, kt * B:(kt + 1) * B],
                                rhs=wt[:, i * segw + nb:i * segw + nb + nw_],
                                start=False, stop=(kt == KT - 1),
                            )
                part = sb.tile([B, segw], f32, tag=f"part{s}")
                nc.vector.tensor_copy(part[:], feat_ps[:])
                nc.gpsimd.dma_start(out=ar_in[s][:], in_=part[:])
                nc.gpsimd.collective_compute(
                    "AllReduce", ADD, replica_groups=rg,
                    ins=[ar_in[s][:]], outs=[ar_out[s][:]],
                )
                # read back the summed segment, transpose, project branch 0
                fseg = sb.tile([B, segw], f32, tag=f"fseg{s}")
                nc.sync.dma_start(out=fseg[:], in_=ar_out[s][:])
                nds = segw // 128
                tp_ps = psT.tile([128, nds * 64], f32, tag="tp")
                for dd in range(nds):
                    nc.tensor.transpose(
                        tp_ps[:, dd * 64:(dd + 1) * 64],
                        fseg[:, dd * 128:(dd + 1) * 128], ident[:])
                pfT = sb.tile([128, nds * B], f32r, tag=f"pfT{s}")
                nc.vector.tensor_copy(pfT[:], tp_ps[:])
                for dd in range(nds):
                    d = b0 // 128 + dd
                    nc.tensor.matmul(
                        qkv0_ps[:, :],
                        lhsT=pfT[:, dd * B:(dd + 1) * B],
                        rhs=headw[:, d * 416:(d + 1) * 416],
                        start=(d == 0), stop=(d == 7),
                    )
            qkv0 = sb.tile([B, 416], f32, tag="qkv0")
            nc.vector.tensor_copy(qkv0[:], qkv0_ps[:])

            qkv = [qkv0, p12[0], p12[1]]
            # col layout per branch: q 0:128 | k 128:256 | red 256:288 | z 288:416

            # ---------- score partials (critical path to AG2) ----------
            ag2s = sb.tile([B, 9], f32, tag="ag2s")
            scr = sb.tile([B, 128], f32, tag="scr")
            scr9 = sb.tile([B, 9 * 128], f32, tag="scr9")
            for n in range(3):
                for m in range(3):
                    j = n * 3 + m
                    nc.vector.scalar_tensor_tensor(
                        out=scr9[:, j * 128:(j + 1) * 128],
                        in0=qkv[n][:, 0:128],
                        scalar=1.0 / 32.0, in1=qkv[m][:, 128:256],
                        op0=MUL, op1=MUL)
            nc.vector.tensor_reduce(
                ag2s[:].rearrange("p (j o) -> p j o", o=1),
                scr9[:].rearrange("p (j d) -> p j d", j=9),
                axis=mybir.AxisListType.X, op=ADD)

            nc.gpsimd.dma_start(out=ag2_in[:], in_=ag2s[:])
            nc.gpsimd.collective_compute(
                "AllGather", mybir.AluOpType.bypass, replica_groups=rg,
                ins=[ag2_in[:]], outs=[ag2_out[:]],
            )

            # ---------- moment branch (overlaps the AllGather) ----------
            mom = sb.tile([B, 128], f32, tag="mom")
            r_ = [qkv[n][:, 256:288] for n in range(3)]

            def lincomb(dst, coef):
                nc.vector.tensor_scalar_mul(dst, r_[0], float(coef[0]))
                nc.vector.scalar_tensor_tensor(
                    out=dst, in0=r_[1], scalar=float(coef[1]), in1=dst,
                    op0=MUL, op1=ADD)
                nc.vector.scalar_tensor_tensor(
                    out=dst, in0=r_[2], scalar=float(coef[2]), in1=dst,
                    op0=MUL, op1=ADD)

            lincomb(mom[:, 0:32], _MCOEF)
            xc = []
            for kk in range(3):
                t = sb.tile([B, 32], f32, tag=f"xc{kk}")
                lincomb(t[:], _ACOEF[kk])
                xc.append(t)
            sumsq = sb.tile([B, 32], f32, tag="sumsq")
            sum3 = sb.tile([B, 32], f32, tag="sum3")
            sum4 = sb.tile([B, 32], f32, tag="sum4")
            tmp = sb.tile([B, 32], f32, tag="mtmp")
            sq = []
            for kk in range(3):
                s2 = sb.tile([B, 32], f32, tag=f"sq{kk}")
                nc.scalar.square(s2[:], xc[kk][:])
                sq.append(s2)
            nc.vector.tensor_add(sumsq[:], sq[0][:], sq[1][:])
            nc.vector.tensor_add(sumsq[:], sumsq[:], sq[2][:])
            nc.vector.tensor_mul(sum3[:], sq[0][:], xc[0][:])
            nc.vector.tensor_mul(tmp[:], sq[1][:], xc[1][:])
            nc.vector.tensor_add(sum3[:], sum3[:], tmp[:])
            nc.vector.tensor_mul(tmp[:], sq[2][:], xc[2][:])
            nc.vector.tensor_add(sum3[:], sum3[:], tmp[:])
            nc.scalar.square(sum4[:], sq[0][:])
            nc.scalar.square(tmp[:], sq[1][:])
            nc.vector.tensor_add(sum4[:], sum4[:], tmp[:])
            nc.scalar.square(tmp[:], sq[2][:])
            nc.vector.tensor_add(sum4[:], sum4[:], tmp[:])

            nc.vector.tensor_scalar_mul(mom[:, 32:64], sumsq[:], 1.0 / 3.0)
            veps = sb.tile([B, 32], f32, tag="veps")
            nc.vector.tensor_scalar(veps[:], sumsq[:], 1.0 / 3.0, EPS,
                                    MUL, ADD)
            std = sb.tile([B, 32], f32, tag="std")
            nc.scalar.sqrt(std[:], veps[:])
            d3 = sb.tile([B, 32], f32, tag="d3")
            nc.vector.tensor_mul(d3[:], std[:], veps[:])
            nc.vector.tensor_scalar(d3[:], d3[:], 3.0, 3.0 * EPS, MUL, ADD)
            rec = sb.tile([B, 32], f32, tag="rec")
            nc.vector.reciprocal(rec[:], d3[:])
            nc.vector.tensor_mul(mom[:, 64:96], sum3[:], rec[:])
            d4 = sb.tile([B, 32], f32, tag="d4")
            nc.vector.tensor_mul(d4[:], veps[:], veps[:])
            nc.vector.tensor_scalar(d4[:], d4[:], 3.0, 3.0 * EPS, MUL, ADD)
            nc.vector.reciprocal(rec[:], d4[:])
            nc.vector.tensor_mul(mom[:, 96:128], sum4[:], rec[:])

            # ---------- gather scores, softmax, attention mix ----------
            ag2g = sb.tile([B, 9 * NCORES], f32, tag="ag2g")
            nc.sync.dma_start(
                out=ag2g[:].rearrange("p (f r) -> p f r", r=NCORES),
                in_=ag2_out.rearrange("(r p) f -> p f r", p=B),
            )
            red2 = sb.tile([B, 9], f32, tag="red2")
            nc.vector.tensor_reduce(
                red2[:],
                ag2g[:].rearrange("p (f r) -> p f r", r=NCORES),
                axis=mybir.AxisListType.X, op=ADD)

            wt3 = sb.tile([B, 9], f32, tag="wt3")
            negmx = sb.tile([B, 3], f32, tag="negmx")
            sume = sb.tile([B, 3], f32, tag="sume")
            rn = sb.tile([B, 3], f32, tag="rn")
            etile = sb.tile([B, 9], f32, tag="etile")
            nc.vector.tensor_reduce(negmx[:],
                                    red2[:].rearrange("p (n m) -> p n m", n=3),
                                    axis=mybir.AxisListType.X, op=MAX)
            nc.vector.tensor_scalar_mul(negmx[:], negmx[:], -1.0)
            for n in range(3):
                nc.scalar.activation(etile[:, n * 3:(n + 1) * 3],
                                     red2[:, n * 3:(n + 1) * 3], Act.Exp,
                                     bias=negmx[:, n:n + 1],
                                     scale=1.0,
                                     accum_out=sume[:, n:n + 1])
            nc.vector.tensor_scalar_mul(sume[:], sume[:], 3.0)
            nc.vector.reciprocal(rn[:], sume[:])
            for n in range(3):
                nc.vector.tensor_scalar(wt3[:, n * 3:(n + 1) * 3],
                                        etile[:, n * 3:(n + 1) * 3],
                                        rn[:, n:n + 1], None, MUL)
            wbar = sb.tile([B, 3], f32, tag="wbar")
            nc.vector.tensor_reduce(
                wbar[:],
                wt3[:].rearrange("p (n m) -> p m n", n=3),
                axis=mybir.AxisListType.X, op=ADD)

            ao = sb.tile([B, 128], f32, tag="ao")
            nc.vector.tensor_scalar(ao[:], qkv[0][:, 288:416],
                                    wbar[:, 0:1], None, MUL)
            nc.vector.scalar_tensor_tensor(
                out=ao[:], in0=qkv[1][:, 288:416], scalar=wbar[:, 1:2],
                in1=ao[:], op0=MUL, op1=ADD)
            nc.vector.scalar_tensor_tensor(
                out=ao[:], in0=qkv[2][:, 288:416], scalar=wbar[:, 2:3],
                in1=ao[:], op0=MUL, op1=ADD)

            # ---------- partial sums for host-side fc/l2norm ----------
            outp = sb.tile([B, 4], f32, tag="outp")
            scr2 = sb.tile([B, 128], f32, tag="scr2")
            nc.vector.tensor_mul(scr2[:], ao[:], fcx[:, 0:128])
            nc.vector.tensor_reduce(outp[:, 0:1], scr2[:],
                                    axis=mybir.AxisListType.X, op=ADD)
            nc.scalar.activation(scr2[:], ao[:], Act.Square,
                                 accum_out=outp[:, 1:2])
            nc.vector.tensor_mul(scr[:], mom[:], fcx[:, 128:256])
            nc.vector.tensor_reduce(outp[:, 2:3], scr[:],
                                    axis=mybir.AxisListType.X, op=ADD)
            nc.scalar.activation(scr[:], mom[:], Act.Square,
                                 accum_out=outp[:, 3:4])
            nc.sync.dma_start(out=out_d[:], in_=outp[:])

            if dbg_d is not None:
                nc.sync.dma_start(out=dbg_d[:, 0:416], in_=qkv0[:])
                nc.sync.dma_start(out=dbg_d[:, 416:425], in_=red2[:])
                nc.sync.dma_start(out=dbg_d[:, 425:428], in_=wbar[:])
                nc.sync.dma_start(out=dbg_d[:, 428:556], in_=ao[:])

    nc.compile()
    _prog_cache = nc
    return nc


def _make_in_maps(x, feat_orig, noise_feat, W_clip, W_red, Wq, Wk, Wv, Wo,
                  fc_w, fc_b):
    import ml_dtypes
    bf = ml_dtypes.bfloat16

    x = np.asarray(x, dtype=np.float32)
    feat_orig = np.asarray(feat_orig, dtype=np.float32)
    noise_feat = np.asarray(noise_feat, dtype=np.float32)
    W_clip = np.asarray(W_clip, dtype=np.float32)
    W_red = np.asarray(W_red, dtype=np.float32)
    fc_w = np.asarray(fc_w, dtype=np.float32)

    idx = _shuffle_idx()
    xs = np.take_along_axis(x.reshape(B, -1), idx, axis=1)  # (64, 150528)
    xs4 = xs.reshape(B, NCORES, KT, 128)
    xsT = np.ascontiguousarray(xs4.transpose(1, 3, 2, 0)).reshape(
        NCORES, 128, KT * B)
    xh = xsT.astype(bf)
    xl = (xsT - xh.astype(np.float32)).astype(bf)

    fton = np.empty((128, 8, 128), dtype=np.float32)
    fton[:, :, 0:64] = feat_orig.T.reshape(8, 128, B).transpose(1, 0, 2)
    fton[:, :, 64:128] = noise_feat.T.reshape(8, 128, B).transpose(1, 0, 2)
    fton = fton.reshape(128, 8 * 128)

    Wvo = (np.asarray(Wv, np.float64) @ np.asarray(Wo, np.float64)).astype(
        np.float32)

    in_maps = []
    for c in range(NCORES):
        headw = np.concatenate([
            np.asarray(Wq, np.float32)[:, c * DS:(c + 1) * DS],
            np.asarray(Wk, np.float32)[:, c * DS:(c + 1) * DS],
            W_red[:, c * RS:(c + 1) * RS],
            Wvo[:, c * DS:(c + 1) * DS],
        ], axis=1)  # (1024, 416) f32 (consumed as f32r)
        fcwa = fc_w[c * DS:(c + 1) * DS, 0]
        fcwb = fc_w[D + np.arange(4)[:, None] * 256
                    + c * RS + np.arange(RS)[None, :], 0].reshape(-1)
        fcx = np.empty((B, 257), dtype=np.float32)
        fcx[:, 0:128] = fcwa[None, :]
        fcx[:, 128:256] = fcwb[None, :]
        fcx[:, 256] = 0.0
        in_maps.append({
            "xh": xh[c],
            "xl": xl[c],
            "w": W_clip[c * KC:(c + 1) * KC],
            "fton": fton,
            "headw": headw,
            "fcx": fcx,
        })
    return in_maps


def kernel(**inputs) -> np.ndarray:
    from concourse.bass_utils import run_bass_kernel_spmd
    fc_b = np.asarray(inputs["fc_b"], dtype=np.float32)
    in_maps = _make_in_maps(**inputs)
    nc = _build_program()
    for attempt in range(3):
        res = run_bass_kernel_spmd(nc, in_maps, core_ids=list(range(NCORES)))
        # unshard: sum per-core fc/l2norm partials, finish the scalar math
        P = np.zeros((B, 4), dtype=np.float64)
        for c in range(NCORES):
            P += res.results[c]["out"].astype(np.float64)
        nA = np.maximum(np.sqrt(np.maximum(P[:, 1], 0.0)), 1e-12)
        nB = np.maximum(np.sqrt(np.maximum(P[:, 3], 0.0)), 1e-12)
        logits = P[:, 0] / nA + P[:, 2] / nB + fc_b[0]
        # sanity gate: partials bounded, norms positive, logits finite/small
        ok = (np.all(np.isfinite(P)) and np.all(np.abs(P) < 1e8)
              and np.all(P[:, 1] > 0) and np.all(P[:, 3] > 0)
              and np.all(np.isfinite(logits))
              and np.all(np.abs(logits) < 1e4))
        if ok:
            break
    return logits.astype(np.float32).reshape(B, 1)
